# revision 8
# baseline (speedup 1.0000x reference)
"""Trainium2 Bass kernel for nn_AttPCB (grouped 6-token attention block).

Math (per sample n):
  x   = parts_feat[n,:,:,0]                      # [C=2048, P=6]
  q/k/v = W x + b                                # [D=512, 6]
  per group g (8 groups of 64 channels):
    qk = (Qg^T Kg) / 8 ; attn = softmax(qk, -1)  # [6, 6]
    out_g = Vg @ attn^T                          # [64, 6]
  o  = wo @ out + bo                             # [2048, 6]
  ret = x + o

Distribution: pure data parallel over N=4096 samples across 8 cores
(512 samples/core).  Weights are replicated (host pre-transposes them
so the device only streams x).

On-device dataflow per 128-sample block (tokens are (p, n) pairs,
p-major; 768 tokens per block):
  1. DMA x in natural layout [n, (c,p)] (4 c-quarter tiles).
  2. PE-transpose x -> x_t[cc] = [c:128, (p,n):768] bf16.
  3. QKV projections: psum[tok,512] += x_t[cc](stationary) @ wT (moving),
     + bias via a K=1 ones-row matmul; copied to SBUF bf16 (ScalarE).
  4. Attention on VectorE in token-major layout (partition = n):
     products + grouped reductions with strided/broadcast APs,
     softmax via DVE reduce + ScalarE exp.
  5. PE-transpose out^T back to d-major, output projection on PE
     (out stationary, wo^T moving) + bo via ones-row matmul.
  6. Residual add (DVE) in place into the fp32 x tile, DMA out.
"""

import numpy as np
import ml_dtypes

N_FULL = 4096
C = 2048
P = 6
D = 512
G = 8
FD = 64
NCORES = 8
NB = 128          # samples per block
CP = C * P        # 12288
QF = CP // 4      # free elems per c-quarter tile (3072)
TOK = NB * P      # tokens per block (768)

_CACHE = {}


def _build(ns, reps=1):
    """Build the Bass/Tile program for one core processing ns samples."""
    from contextlib import ExitStack

    import concourse.bass as bass
    import concourse.tile as tile
    import concourse.mybir as mybir
    from concourse import bacc
    from concourse.bass_types import AP
    from concourse.masks import make_identity

    f32 = mybir.dt.float32
    bf16 = mybir.dt.bfloat16
    MULT = mybir.AluOpType.mult
    ADD = mybir.AluOpType.add
    MAX = mybir.AluOpType.max
    SUB = mybir.AluOpType.subtract
    AX = mybir.AxisListType.X
    COPY = mybir.ActivationFunctionType.Copy
    EXP = mybir.ActivationFunctionType.Exp

    assert ns % NB == 0
    nblocks = ns // NB

    nc = bacc.Bacc("TRN2", target_bir_lowering=False, debug=False)

    x_d = nc.dram_tensor("x", [ns, CP], f32, kind="ExternalInput")
    wT_d = nc.dram_tensor("wT", [C, 3 * D], bf16, kind="ExternalInput")
    woT_d = nc.dram_tensor("woT", [D, C], bf16, kind="ExternalInput")
    bqkv_d = nc.dram_tensor("bqkv", [1, 3 * D], bf16, kind="ExternalInput")
    bo_d = nc.dram_tensor("bo", [1, C], bf16, kind="ExternalInput")
    out_d = nc.dram_tensor("out", [ns, CP], f32, kind="ExternalOutput")

    def ap(tile_ap, off, dims):
        """Custom access pattern into a tile: dims = [[step,count],...]."""
        return AP(tile_ap.tensor, tile_ap.offset + off, dims)

    with ExitStack() as ctx:
        tc = ctx.enter_context(tile.TileContext(nc))

        # ---- persistent weights / constants ----
        wpool = ctx.enter_context(tc.tile_pool(name="wT", bufs=16))
        wopool = ctx.enter_context(tc.tile_pool(name="woT", bufs=4))
        cpool = ctx.enter_context(tc.tile_pool(name="const", bufs=1))

        wT_sb = [wpool.tile([128, 3 * D], bf16, name="wTsb", tag="wT") for _ in range(16)]
        woT_sb = [wopool.tile([128, C], bf16, name="woTsb", tag="woT") for _ in range(4)]
        for cc in range(16):
            nc.sync.dma_start(wT_sb[cc][:], wT_d.ap()[cc * 128:(cc + 1) * 128, :])
        for dc in range(4):
            nc.sync.dma_start(woT_sb[dc][:], woT_d.ap()[dc * 128:(dc + 1) * 128, :])

        bqkv_sb = cpool.tile([1, 3 * D], bf16, tag="bqkv")
        bo_sb = cpool.tile([1, C], bf16, tag="bo")
        ones_sb = cpool.tile([1, 128], bf16, tag="ones")
        ident = cpool.tile([128, 128], f32, tag="ident")
        nc.sync.dma_start(bqkv_sb[:], bqkv_d.ap()[:, :])
        nc.sync.dma_start(bo_sb[:], bo_d.ap()[:, :])
        nc.gpsimd.memset(ones_sb[:], 1.0)
        make_identity(nc, ident[:])

        # ---- per-block pools ----
        xq_pool = ctx.enter_context(tc.tile_pool(name="xq", bufs=5))
        xt_psum = ctx.enter_context(tc.tile_pool(name="xtps", bufs=2, space="PSUM"))
        xt_pool = ctx.enter_context(tc.tile_pool(name="xt", bufs=16))
        qkv_psum = ctx.enter_context(tc.tile_pool(name="qkvps", bufs=2, space="PSUM"))
        qkv_pool = ctx.enter_context(tc.tile_pool(name="qkv", bufs=2))
        tmp_pool = ctx.enter_context(tc.tile_pool(name="tmp", bufs=1))
        sm_pool = ctx.enter_context(tc.tile_pool(name="sm", bufs=1))
        outT_pool = ctx.enter_context(tc.tile_pool(name="outT", bufs=1))
        ot_psum = ctx.enter_context(tc.tile_pool(name="otps", bufs=2, space="PSUM"))
        od_pool = ctx.enter_context(tc.tile_pool(name="od", bufs=6))
        o_psum = ctx.enter_context(tc.tile_pool(name="ops", bufs=2, space="PSUM"))

        for b in range(nblocks * reps):
            b = b % nblocks
            r0 = b * NB
            # -- 1. input DMA (c-quarters, natural layout) --
            xq = [xq_pool.tile([128, QF], f32, name="xq", tag="xq") for _ in range(4)]
            for qi in range(4):
                nc.sync.dma_start(xq[qi][:], x_d.ap()[r0:r0 + NB, qi * QF:(qi + 1) * QF])

            # -- 2. transpose x -> x_t [c:128, (p,n):768] bf16 --
            xt = [xt_pool.tile([128, TOK], bf16, name="xt", tag="xt") for _ in range(16)]
            for cc in range(16):
                qi, ci = divmod(cc, 4)
                for half in range(2):
                    ps = xt_psum.tile([128, 3 * NB], f32, tag="xtps")
                    for pp in range(3):
                        p = half * 3 + pp
                        src = ap(xq[qi][:], ci * 128 * P + p, [[QF, 128], [P, 128]])
                        nc.tensor.transpose(ps[:, pp * 128:(pp + 1) * 128], src, ident[:])
                    nc.scalar.activation(
                        xt[cc][:, half * 3 * NB:(half + 1) * 3 * NB], ps[:], COPY)

            # -- 3. QKV projections (token-major out: [tok, d]) --
            q_all = qkv_pool.tile([128, 6 * D], bf16, tag="q")
            k_all = qkv_pool.tile([128, 6 * D], bf16, tag="k")
            v_all = qkv_pool.tile([128, 6 * D], bf16, tag="v")
            for p in range(P):
                for j, dst in enumerate((q_all, k_all, v_all)):
                    ps = qkv_psum.tile([128, D], f32, tag="qkvps")
                    for cc in range(16):
                        nc.tensor.matmul(
                            ps[:],
                            lhsT=xt[cc][:, p * 128:(p + 1) * 128],
                            rhs=wT_sb[cc][:, j * D:(j + 1) * D],
                            start=(cc == 0), stop=False)
                    nc.tensor.matmul(
                        ps[:], lhsT=ones_sb[:, 0:128],
                        rhs=bqkv_sb[:, j * D:(j + 1) * D],
                        start=False, stop=True)
                    nc.scalar.activation(dst[:, p * D:(p + 1) * D], ps[:], COPY)

            # -- 4. attention (token-major, partition = n) --
            # qk[n,(p,g,q')] = sum_f q[n,(p,g,f)] * k[n,(q',g,f)]
            qk = sm_pool.tile([128, P * G * P], f32, tag="qk")  # [128, 288]
            for p in range(P):
                for h in range(2):  # q' half
                    tmp = tmp_pool.tile([128, 3 * D], bf16, tag="tmp")  # (q'h,g,f)
                    in0 = ap(q_all[:], p * D,
                             [[6 * D, 128], [0, 3], [FD, G], [1, FD]])
                    in1 = ap(k_all[:], h * 3 * D,
                             [[6 * D, 128], [D, 3], [FD, G], [1, FD]])
                    o3 = ap(tmp[:], 0, [[3 * D, 128], [D, 3], [FD, G], [1, FD]])
                    nc.vector.tensor_tensor(o3, in0, in1, op=MULT)
                    nc.vector.tensor_reduce(
                        ap(qk[:], p * 48 + h * 3, [[288, 128], [1, 3], [6, G]]),
                        ap(tmp[:], 0, [[3 * D, 128], [D, 3], [FD, G], [1, FD]]),
                        axis=AX, op=ADD)

            # softmax over q' (innermost of (p,g,q') layout), logits scaled 1/8
            rmax = sm_pool.tile([128, 48], f32, tag="rmax")
            attn = sm_pool.tile([128, 288], bf16, tag="attn")
            qk3 = ap(qk[:], 0, [[288, 128], [6, 48], [1, 6]])
            nc.vector.tensor_reduce(rmax[:], qk3, axis=AX, op=MAX)
            nc.vector.scalar_tensor_tensor(
                qk3, qk3, 1.0,
                ap(rmax[:], 0, [[48, 128], [1, 48], [0, 6]]),
                op0=MULT, op1=SUB)
            nc.scalar.activation(qk[:], qk[:], EXP, scale=0.125)
            ssum = sm_pool.tile([128, 48], f32, tag="ssum")
            nc.vector.tensor_reduce(ssum[:], qk3, axis=AX, op=ADD)
            recip = ssum
            nc.vector.reciprocal(recip[:], ssum[:])
            nc.vector.tensor_tensor(
                ap(attn[:], 0, [[288, 128], [6, 48], [1, 6]]), qk3,
                ap(recip[:], 0, [[48, 128], [1, 48], [0, 6]]), op=MULT)

            # out^T[n,(g,f)] per p = sum_q' attn[n,(p,g,q')] * v[n,(q',g,f)]
            # then transpose to d-major, output-project, residual-add.
            od = []
            for p in range(P):
                outT = outT_pool.tile([128, D], f32, name="outT", tag="outT")
                for h in range(2):  # g half
                    tmp2 = tmp_pool.tile([128, 3 * D], bf16, tag="tmp")  # (q',gh,f)
                    a0 = ap(attn[:], p * 48 + h * 4 * P,
                            [[288, 128], [1, 6], [6, 4], [0, FD]])
                    v0 = ap(v_all[:], h * 4 * FD,
                            [[6 * D, 128], [D, 6], [FD, 4], [1, FD]])
                    t0 = ap(tmp2[:], 0, [[3 * D, 128], [256, 6], [FD, 4], [1, FD]])
                    nc.vector.tensor_tensor(t0, a0, v0, op=MULT)
                    nc.vector.tensor_reduce(
                        ap(outT[:], h * 4 * FD, [[D, 128], [FD, 4], [1, FD]]),
                        ap(tmp2[:], 0, [[3 * D, 128], [FD, 4], [1, FD], [256, 6]]),
                        axis=AX, op=ADD)

                # -- 5. transpose out^T -> d-major bf16 --
                ps = ot_psum.tile([128, D], f32, tag="otps")
                for dc in range(4):
                    nc.tensor.transpose(
                        ps[:, dc * 128:(dc + 1) * 128],
                        outT[:, dc * 128:(dc + 1) * 128],
                        ident[:])
                od_p = od_pool.tile([128, D], bf16, name="od", tag="od")
                nc.scalar.activation(od_p[:], ps[:], COPY)

                od.append(od_p)

            # -- 6. output projection + bias + residual (in place into xq) --
            for co in range(4):
                for p in range(P):
                    pso = o_psum.tile([128, D], f32, tag="ops")
                    for dc in range(4):
                        nc.tensor.matmul(
                            pso[:],
                            lhsT=od[p][:, dc * 128:(dc + 1) * 128],
                            rhs=woT_sb[dc][:, co * D:(co + 1) * D],
                            start=(dc == 0), stop=False)
                    nc.tensor.matmul(
                        pso[:], lhsT=ones_sb[:, 0:128],
                        rhs=bo_sb[:, co * D:(co + 1) * D],
                        start=False, stop=True)
                    xsl = ap(xq[co][:], p, [[QF, 128], [P, D]])
                    nc.vector.tensor_tensor(xsl, pso[:], xsl, op=ADD)
                # -- 7. store (releases this xq slot for the next block) --
                nc.sync.dma_start(
                    out_d.ap()[r0:r0 + NB, co * QF:(co + 1) * QF], xq[co][:])

    nc.compile()
    return nc


def get_program(ns, reps=1):
    key = (ns, reps)
    if key not in _CACHE:
        _CACHE[key] = _build(ns, reps)
    return _CACHE[key]


def _host_prep(inputs):
    bf = ml_dtypes.bfloat16
    wq = np.asarray(inputs["wq"], np.float32)
    wk = np.asarray(inputs["wk"], np.float32)
    wv = np.asarray(inputs["wv"], np.float32)
    wo = np.asarray(inputs["wo"], np.float32)
    wT = np.ascontiguousarray(
        np.concatenate([wq.T, wk.T, wv.T], axis=1)).astype(bf)      # [C, 3D]
    woT = np.ascontiguousarray(np.asarray(wo).T).astype(bf)          # [D, C]
    bqkv = np.concatenate(
        [np.asarray(inputs["bq"], np.float32),
         np.asarray(inputs["bk"], np.float32),
         np.asarray(inputs["bv"], np.float32)]).reshape(1, 3 * D).astype(bf)
    bo = np.asarray(inputs["bo"], np.float32).reshape(1, C).astype(bf)
    return wT, woT, bqkv, bo


def kernel(**inputs):
    from concourse.bass_utils import run_bass_kernel_spmd

    x = np.asarray(inputs["parts_feat"], np.float32)
    n_total = x.shape[0]
    xs = np.ascontiguousarray(x.reshape(n_total, CP))
    ns = n_total // NCORES
    wT, woT, bqkv, bo = _host_prep(inputs)

    nc = get_program(ns)
    in_maps = []
    for i in range(NCORES):
        in_maps.append({
            "x": np.ascontiguousarray(xs[i * ns:(i + 1) * ns]),
            "wT": wT, "woT": woT, "bqkv": bqkv, "bo": bo,
        })
    res = run_bass_kernel_spmd(nc, in_maps, core_ids=list(range(NCORES)))
    out = np.concatenate([r["out"] for r in res.results], axis=0)
    return out.reshape(x.shape).astype(np.float32)


# revision 39
# speedup vs baseline: 45.7365x; 45.7365x over previous
"""Trainium2 Bass kernel for nn_AttPCB (grouped 6-token attention block).

Math (per sample n):
  x   = parts_feat[n,:,:,0]                      # [C=2048, P=6]
  q/k/v = W x + b                                # [D=512, 6]
  per group g (8 groups of 64 channels):
    qk = (Qg^T Kg) / 8 ; attn = softmax(qk, -1)  # [6, 6]
    out_g = Vg @ attn^T                          # [64, 6]
  o  = wo @ out + bo                             # [2048, 6]
  ret = x + o

Distribution: pure data parallel over N=4096 samples across 8 cores
(512 samples/core).  Weights are replicated (host pre-transposes them
so the device only streams x).

On-device dataflow, 3-stage software pipeline over 128-sample blocks
(tokens are (p, n) pairs, p-major; 768 tokens per block), each stage
trailing the previous by one block so PE never waits on same-block DVE:
  head  (PE): SWDGE DMA casts x fp32->bf16 in flight; bf16 PE transposes
        to x_t[cc] = [c:128, (p,n):768]; QKV projections with x_t chunks
        stationary (q/k/v share each LDWEIGHTS) + q-bias via a K=1
        ones-row matmul (k-bias is softmax-invariant and dropped; v-bias
        is folded into bo on the host).  PSUM -> SBUF bf16 on ScalarE.
  attn  (DVE): qk products as 2x-mode bf16 multiplies + log2 add-trees
        over f; softmax over q' without max-subtraction (logits bounded);
        attn*v with a ScalarE broadcast-expand so the multiply stays in
        2x mode; PE-transpose out^T to d-major (od).
  out   (PE): output projection with od stationary over co pairs + bo
        ones-row matmul; residual add (DVE) into a freshly re-read fp32
        x tile; contiguous stores.
LDWEIGHTS deduplication runs as a post-pass before compile.
"""

import numpy as np
import ml_dtypes

N_FULL = 4096
C = 2048
P = 6
D = 512
G = 8
FD = 64
NCORES = 8
NB = 128          # samples per block
CP = C * P        # 12288
QF = CP // 4      # free elems per c-quarter tile (3072)
TOK = NB * P      # tokens per block (768)

_CACHE = {}


def _build(ns, reps=1):
    """Build the Bass/Tile program for one core processing ns samples."""
    from contextlib import ExitStack

    import concourse.bass as bass
    import concourse.tile as tile
    import concourse.mybir as mybir
    from concourse import bacc
    from concourse.bass_types import AP
    from concourse.masks import make_identity

    f32 = mybir.dt.float32
    bf16 = mybir.dt.bfloat16
    MULT = mybir.AluOpType.mult
    ADD = mybir.AluOpType.add
    MAX = mybir.AluOpType.max
    SUB = mybir.AluOpType.subtract
    AX = mybir.AxisListType.X
    COPY = mybir.ActivationFunctionType.Copy
    EXP = mybir.ActivationFunctionType.Exp

    assert ns % NB == 0
    nblocks = ns // NB

    nc = bacc.Bacc("TRN2", target_bir_lowering=False, debug=False)

    x_d = nc.dram_tensor("x", [ns, CP], f32, kind="ExternalInput")
    wT_d = nc.dram_tensor("wT", [C, 3 * D], bf16, kind="ExternalInput")
    woT_d = nc.dram_tensor("woT", [D, C], bf16, kind="ExternalInput")
    bq_d = nc.dram_tensor("bq", [1, D], bf16, kind="ExternalInput")
    bo_d = nc.dram_tensor("bo", [1, C], bf16, kind="ExternalInput")
    out_d = nc.dram_tensor("out", [ns, CP], f32, kind="ExternalOutput")

    def ap(tile_ap, off, dims):
        """Custom access pattern into a tile: dims = [[step,count],...]."""
        return AP(tile_ap.tensor, tile_ap.offset + off, dims)

    with ExitStack() as ctx:
        tc = ctx.enter_context(tile.TileContext(nc))

        # ---- persistent weights / constants ----
        wpool = ctx.enter_context(tc.tile_pool(name="wT", bufs=16))
        wopool = ctx.enter_context(tc.tile_pool(name="woT", bufs=4))
        cpool = ctx.enter_context(tc.tile_pool(name="const", bufs=1))

        wT_sb = [wpool.tile([128, 3 * D], bf16, name="wTsb", tag="wT") for _ in range(16)]
        woT_sb = [wopool.tile([128, C], bf16, name="woTsb", tag="woT") for _ in range(4)]

        def load_weights():
            # emitted after block 0's x DMAs so x streams in first and the
            # weight loads overlap the x transposes
            for cc in range(16):
                nc.sync.dma_start(wT_sb[cc][:], wT_d.ap()[cc * 128:(cc + 1) * 128, :])
            for dc in range(4):
                nc.sync.dma_start(woT_sb[dc][:], woT_d.ap()[dc * 128:(dc + 1) * 128, :])

        bq_sb = cpool.tile([1, D], bf16, tag="bq")
        bo_sb = cpool.tile([1, C], bf16, tag="bo")
        ones_sb = cpool.tile([1, 128], bf16, tag="ones")
        identb = cpool.tile([128, 128], bf16, tag="identb")
        nc.sync.dma_start(bq_sb[:], bq_d.ap()[:, :])
        nc.sync.dma_start(bo_sb[:], bo_d.ap()[:, :])
        nc.gpsimd.memset(ones_sb[:], 1.0)
        make_identity(nc, identb[:])

        # ---- per-block pools ----
        xq_pool = ctx.enter_context(tc.tile_pool(name="xq", bufs=2))
        xr_pool = ctx.enter_context(tc.tile_pool(name="xr", bufs=3))
        sh_psum = ctx.enter_context(tc.tile_pool(name="shps", bufs=4, space="PSUM"))
        xt_psum = sh_psum
        qkv_psum = sh_psum
        ot_psum = sh_psum
        xt_pool = ctx.enter_context(tc.tile_pool(name="xt", bufs=16))
        qkv_pool = ctx.enter_context(tc.tile_pool(name="qkv", bufs=2))
        tmp_pool = ctx.enter_context(tc.tile_pool(name="tmp", bufs=3))
        sm_pool = ctx.enter_context(tc.tile_pool(name="sm", bufs=2))
        outT_pool = ctx.enter_context(tc.tile_pool(name="outT", bufs=1))
        od_pool = ctx.enter_context(tc.tile_pool(name="od", bufs=12))
        o_psum = ctx.enter_context(tc.tile_pool(name="ops", bufs=3, space="PSUM"))

        first_head = [True]

        def emit_head(b):
            """DMA-in + transposes + QKV projections (PE-heavy)."""
            r0 = b * NB
            xt = [xt_pool.tile([128, TOK], bf16, name="xt", tag="xt") for _ in range(16)]
            for qi in range(4):
                # SWDGE DMA casts fp32 -> bf16 in flight; bf16 PE transposes
                # run at 1 cyc/row and halve the x SBUF footprint
                xq = xq_pool.tile([128, QF], bf16, name="xq", tag="xq")
                nc.gpsimd.dma_start(xq[:], x_d.ap()[r0:r0 + NB, qi * QF:(qi + 1) * QF])
                if first_head[0]:
                    first_head[0] = False
                    load_weights()
                for ci in range(4):
                    cc = qi * 4 + ci
                    for half in range(2):
                        ps = xt_psum.tile([128, 3 * NB], bf16, name="ps", tag="tps", bufs=2)
                        for pp in range(3):
                            p = half * 3 + pp
                            sap = ap(xq[:], ci * 128 * P + p, [[QF, 128], [P, 128]])
                            nc.tensor.transpose(ps[:, pp * 128:(pp + 1) * 128], sap, identb[:])
                        nc.scalar.activation(
                            xt[cc][:, half * 3 * NB:(half + 1) * 3 * NB], ps[:], COPY)

            q_all = qkv_pool.tile([128, 6 * D], bf16, tag="q")
            k_all = qkv_pool.tile([128, 6 * D], bf16, tag="k")
            v_all = qkv_pool.tile([128, 6 * D], bf16, tag="v")
            for p in range(P):
                pq = qkv_psum.tile([128, D], f32, name="ps", tag="qkvps", bufs=3)
                pk = qkv_psum.tile([128, D], f32, name="ps", tag="qkvps", bufs=3)
                pv = qkv_psum.tile([128, D], f32, name="ps", tag="qkvps", bufs=3)
                nc.tensor.matmul(pq[:], lhsT=ones_sb[:, 0:128], rhs=bq_sb[:],
                                 start=True, stop=False)
                for cc in range(16):
                    lw = xt[cc][:, p * 128:(p + 1) * 128]
                    last = cc == 15
                    nc.tensor.matmul(pq[:], lhsT=lw, rhs=wT_sb[cc][:, 0:D],
                                     start=False, stop=last)
                    nc.tensor.matmul(pk[:], lhsT=lw, rhs=wT_sb[cc][:, D:2 * D],
                                     start=(cc == 0), stop=last)
                    nc.tensor.matmul(pv[:], lhsT=lw, rhs=wT_sb[cc][:, 2 * D:3 * D],
                                     start=(cc == 0), stop=last)
                nc.scalar.activation(q_all[:, p * D:(p + 1) * D], pq[:], COPY)
                nc.scalar.activation(k_all[:, p * D:(p + 1) * D], pk[:], COPY)
                nc.scalar.activation(v_all[:, p * D:(p + 1) * D], pv[:], COPY)
            return q_all, k_all, v_all

        def emit_attn(b, q_all, k_all, v_all):
            """Attention stage (DVE-heavy) + out-transpose to d-major."""
            qk = sm_pool.tile([128, P * G * P], f32, tag="qk")  # [128, 288]
            for p in range(P):
                for h in range(2):  # q' half
                    tmp = tmp_pool.tile([128, 3 * D], bf16, tag="tmp")  # (q'h,g,f)
                    in0 = ap(q_all[:], p * D,
                             [[6 * D, 128], [0, 3], [FD, G], [1, FD]])
                    in1 = ap(k_all[:], h * 3 * D,
                             [[6 * D, 128], [D, 3], [FD, G], [1, FD]])
                    o3 = ap(tmp[:], 0, [[3 * D, 128], [D, 3], [FD, G], [1, FD]])
                    nc.vector.tensor_tensor(o3, in0, in1, op=MULT)
                    # log2 add-tree over f (bf16 2x-mode TT beats 1x reduce)
                    w = FD
                    while w > 2:
                        w //= 2
                        nc.vector.tensor_tensor(
                            ap(tmp[:], 0, [[3 * D, 128], [FD, 24], [1, w]]),
                            ap(tmp[:], 0, [[3 * D, 128], [FD, 24], [1, w]]),
                            ap(tmp[:], w, [[3 * D, 128], [FD, 24], [1, w]]),
                            op=ADD)
                    nc.vector.tensor_tensor(
                        ap(qk[:], p * 48 + h * 3, [[288, 128], [1, 3], [6, G]]),
                        ap(tmp[:], 0, [[3 * D, 128], [D, 3], [FD, G]]),
                        ap(tmp[:], 1, [[3 * D, 128], [D, 3], [FD, G]]),
                        op=ADD)

            # softmax over q' (innermost of (p,g,q') layout), logits scaled
            # 1/8.  No max-subtraction: |qk/8| stays O(5) for N(0,1) inputs,
            # far inside fp32 exp range, and the normalization divides any
            # common factor out exactly.
            attn = sm_pool.tile([128, 288], bf16, tag="attn")
            qk3 = ap(qk[:], 0, [[288, 128], [6, 48], [1, 6]])
            nc.scalar.activation(qk[:], qk[:], EXP, scale=0.125)
            ssum = sm_pool.tile([128, 48], f32, tag="ssum")
            nc.vector.tensor_reduce(ssum[:], qk3, axis=AX, op=ADD)
            recip = ssum
            nc.vector.reciprocal(recip[:], ssum[:])
            nc.vector.tensor_tensor(
                ap(attn[:], 0, [[288, 128], [6, 48], [1, 6]]), qk3,
                ap(recip[:], 0, [[48, 128], [1, 48], [0, 6]]), op=MULT)

            # out^T[n,(g,f)] per p = sum_q' attn[n,(p,g,q')] * v[n,(q',g,f)]
            od = []
            for p in range(P):
                outT = outT_pool.tile([128, D], bf16, name="outT", tag="outT")
                for h in range(2):  # g half
                    tmp2 = tmp_pool.tile([128, 3 * D], bf16, tag="tmp")  # (q',gh,f)
                    a0 = ap(attn[:], p * 48 + h * 4 * P,
                            [[288, 128], [1, 6], [6, 4], [0, FD]])
                    v0 = ap(v_all[:], h * 4 * FD,
                            [[6 * D, 128], [D, 6], [FD, 4], [1, FD]])
                    t0 = ap(tmp2[:], 0, [[3 * D, 128], [256, 6], [FD, 4], [1, FD]])
                    # broadcast-expand attn over f on ScalarE (otherwise the
                    # step-0 input AP forces the DVE multiply into 1x mode)
                    nc.scalar.activation(t0, a0, COPY)
                    nc.vector.tensor_tensor(tmp2[:], tmp2[:], v0, op=MULT)
                    # add-tree over q' (6 planes of 256): (0,1,2)+=(3,4,5);
                    # 0+=2; out = plane0 + plane1 (fp32)
                    nc.vector.tensor_tensor(
                        tmp2[:, 0:768], tmp2[:, 0:768], tmp2[:, 768:1536], op=ADD)
                    nc.vector.tensor_tensor(
                        tmp2[:, 0:256], tmp2[:, 0:256], tmp2[:, 512:768], op=ADD)
                    nc.vector.tensor_tensor(
                        ap(outT[:], h * 4 * FD, [[D, 128], [1, 256]]),
                        tmp2[:, 0:256], tmp2[:, 256:512], op=ADD)

                ps = ot_psum.tile([128, D], bf16, name="ps", tag="tps", bufs=2)
                for dc in range(4):
                    nc.tensor.transpose(
                        ps[:, dc * 128:(dc + 1) * 128],
                        outT[:, dc * 128:(dc + 1) * 128],
                        identb[:])
                od_p = od_pool.tile([128, D], bf16, name="od", tag="od")
                nc.scalar.activation(od_p[:], ps[:], COPY)
                od.append(od_p)
            return od

        def emit_out(b, od):
            """Output projection + bias + residual (into xr, re-read from
            DRAM so x tiles don't pin the pipeline) + store."""
            r0 = b * NB
            xr = {}
            for co in range(4):
                xr[co] = xr_pool.tile([128, QF], f32, name="xr", tag="xr")
                nc.sync.dma_start(
                    xr[co][:], x_d.ap()[r0:r0 + NB, co * QF:(co + 1) * QF])
            for cp in range(2):
                cos = (2 * cp, 2 * cp + 1)
                for p in range(P):
                    pso = {co: o_psum.tile([128, D], f32, name="pso", tag="ops")
                           for co in cos}
                    for co in cos:
                        nc.tensor.matmul(pso[co][:], lhsT=ones_sb[:, 0:128],
                                         rhs=bo_sb[:, co * D:(co + 1) * D],
                                         start=True, stop=False)
                    for dc in range(4):
                        lw = od[p][:, dc * 128:(dc + 1) * 128]
                        for co in cos:
                            nc.tensor.matmul(
                                pso[co][:], lhsT=lw,
                                rhs=woT_sb[dc][:, co * D:(co + 1) * D],
                                start=False, stop=(dc == 3))
                    for co in cos:
                        xsl = ap(xr[co][:], p, [[QF, 128], [P, D]])
                        nc.vector.tensor_tensor(xsl, pso[co][:], xsl, op=ADD)
                for co in cos:
                    nc.sync.dma_start(
                        out_d.ap()[r0:r0 + NB, co * QF:(co + 1) * QF], xr[co][:])

        if reps == 0:
            # timing-baseline null program: same I/O tensors, trivial work
            z = xq_pool.tile([128, QF], bf16, name="xq", tag="xq")
            nc.gpsimd.dma_start(z[:, 0:64], x_d.ap()[0:128, 0:64])
            zf = xr_pool.tile([128, QF], f32, name="xr", tag="xr")
            nc.gpsimd.memset(zf[:, 0:64], 0)
            nc.sync.dma_start(out_d.ap()[0:128, 0:64], zf[:, 0:64])
            load_weights()
            nb_total = 0
        else:
            nb_total = nblocks * reps

        # 3-stage software pipeline: head (PE projections), attention
        # (DVE), output (PE + residual + store), each trailing by one block
        # so no engine waits on same-block work from another engine.
        hcarry = None
        acarry = None
        for i in range(nb_total + 2):
            nxt_h = None
            if i < nb_total:
                nxt_h = (i % nblocks, emit_head(i % nblocks))
            nxt_a = None
            if hcarry is not None:
                hb, h = hcarry
                nxt_a = (hb, emit_attn(hb, *h))
            if acarry is not None:
                ab, od = acarry
                emit_out(ab, od)
            hcarry = nxt_h
            acarry = nxt_a

    _dedupe_ldweights(nc, mybir)
    nc.compile()
    return nc


def _dedupe_ldweights(nc, mybir):
    """Drop InstLdweights whose weights AP is identical to the previous one
    on the PE stream (no intervening transpose, which reloads the array).
    The scheduler places same-lhsT matmuls back to back after the loop
    reordering, so this removes most of the PE-sequencer LDW dispatch cost.
    Waits/updates on a dropped LDW are merged into the following matmul's
    sync_info (multi-wait is legal pre-compile; generate_event_semaphores
    splits them later)."""

    def apkey(a):
        return (str(a.memref), str(a.offset), str(a.ap), str(a.dtype))

    for blk in nc.m.functions[0].blocks:
        insts = blk.instructions
        last = None
        drop = set()
        pending_sync = []
        for idx, ins in enumerate(insts):
            nm = type(ins).__name__
            if nm == "InstLdweights":
                key = (apkey(ins.ins[0]), str(ins.perf_mode),
                       str(ins.is_transpose), str(ins.tile_position))
                if key == last:
                    drop.add(idx)
                    if ins.sync_info is not None:
                        pending_sync.append(ins.sync_info)
                last = key
            elif nm == "InstMatmult":
                if getattr(ins, "is_transpose", False):
                    last = None
                if pending_sync:
                    si = ins.sync_info
                    if si is None:
                        si = mybir.SyncInfo(on_wait=[], on_update=[])
                    for extra in pending_sync:
                        si.on_wait = list(si.on_wait) + list(extra.on_wait)
                        si.on_update = list(si.on_update) + list(extra.on_update)
                    ins.sync_info = si
                    pending_sync = []
        if drop:
            assert not pending_sync
            keep = [i for idx, i in enumerate(insts) if idx not in drop]
            blk.set_instructions_from_list(keep) if hasattr(blk, "set_instructions_from_list") else None
            if not hasattr(blk, "set_instructions_from_list"):
                del insts[:]
                insts.extend(keep)


def get_program(ns, reps=1):
    key = (ns, reps)
    if key not in _CACHE:
        _CACHE[key] = _build(ns, reps)
    return _CACHE[key]


def _host_prep(inputs):
    bf = ml_dtypes.bfloat16
    wq = np.asarray(inputs["wq"], np.float32)
    wk = np.asarray(inputs["wk"], np.float32)
    wv = np.asarray(inputs["wv"], np.float32)
    wo = np.asarray(inputs["wo"], np.float32)
    wT = np.ascontiguousarray(
        np.concatenate([wq.T, wk.T, wv.T], axis=1)).astype(bf)      # [C, 3D]
    woT = np.ascontiguousarray(np.asarray(wo).T).astype(bf)          # [D, C]
    # k-bias is softmax-invariant (adds a row-constant to the logits);
    # v-bias passes through attention unchanged (sum(attn)==1) so it folds
    # into the output-projection bias: bo_eff = bo + wo @ bv.
    bq = np.asarray(inputs["bq"], np.float32).reshape(1, D).astype(bf)
    bo_eff = (np.asarray(inputs["bo"], np.float32)
              + wo.astype(np.float64) @ np.asarray(inputs["bv"], np.float64)
              ).astype(np.float32).reshape(1, C).astype(bf)
    return wT, woT, bq, bo_eff


def kernel(**inputs):
    from concourse.bass_utils import run_bass_kernel_spmd

    x = np.asarray(inputs["parts_feat"], np.float32)
    n_total = x.shape[0]
    xs = np.ascontiguousarray(x.reshape(n_total, CP))
    ns = n_total // NCORES
    wT, woT, bq, bo = _host_prep(inputs)

    nc = get_program(ns)
    in_maps = []
    for i in range(NCORES):
        in_maps.append({
            "x": np.ascontiguousarray(xs[i * ns:(i + 1) * ns]),
            "wT": wT, "woT": woT, "bq": bq, "bo": bo,
        })
    res = run_bass_kernel_spmd(nc, in_maps, core_ids=list(range(NCORES)))
    out = np.concatenate([r["out"] for r in res.results], axis=0)
    return out.reshape(x.shape).astype(np.float32)
